# revision 1
# baseline (speedup 1.0000x reference)
"""Trainium2 Bass kernel for nn_Attention_45749991637079.

Reference computation (per batch b, C=192 channels, 128x128 image, 8 heads):
    qkv  = w_qkv @ x                       (1x1 conv; k-branch unused)
    q,v  = depthwise 3x3 (SAME) of the q/v channel blocks
    qd   = q[:, ::2, ::2]                  (64x64 downsample)
    attn = softmax(l2norm-rows(qd_h) gram * temp)   per head (24x24)
    out  = w_proj @ (attn @ v)             == (w_proj @ blockdiag(attn)) @ v

Sharding: data-parallel over batch; one batch per NeuronCore (8 cores).

Device algorithm per core:
  Phase A (q path): band over image rows; pointwise conv via PE matmuls
    (PSUM) -> padded SBUF band buffer -> 9 "tap" diag-matmuls with shifted
    strided views, PSUM-accumulated -> qd.  Then row norms, scaling,
    PE transposes, per-head gram, softmax, blockdiag(A), WfT = A^T @ WpT.
  Phase B (v path): band over rows; pointwise conv -> padded buffer ->
    9 tap diag-matmuls -> v_dw -> final matmul with WfT -> DMA out.

All weight transposes / diag-tap matrices are precomputed on host.
"""

import numpy as np

C = 192
H = W = 128
HW = H * W
HEADS = 8
CHD = 24
P0, P1 = 128, 64          # channel partition chunks: 0:128 and 128:192
BAND = 16                 # output image rows per band
NB = H // BAND            # 8 bands
PWR = BAND + 2            # pointwise rows computed per band (halo)
PBW = 130                 # padded row width (1 + 128 + 1)
PBSZ = PWR * PBW          # padded band cols per chunk
SUB = 512                 # output subtile cols (4 image rows)
NSUB = BAND * W // SUB    # 4 per band
TAPS = [(di, dj) for di in range(3) for dj in range(3)]

_BUILT = {}


def _build(iters=1):
    import concourse.mybir as mybir
    import concourse.tile as tile
    from concourse import bacc

    f32 = mybir.dt.float32
    f32r = mybir.dt.float32r
    f16 = mybir.dt.float16
    Alu = mybir.AluOpType
    Act = mybir.ActivationFunctionType
    Ax = mybir.AxisListType

    nc = bacc.Bacc(
        "TRN2", target_bir_lowering=False, debug=False,
        enable_asserts=False, num_devices=8,
    )

    # DRAM I/O (per-core shapes)
    xb = nc.dram_tensor("xb", (C, HW), f16, kind="ExternalInput").ap()
    wq = nc.dram_tensor("wq", (P0, 576), f16, kind="ExternalInput").ap()
    wv = nc.dram_tensor("wv", (P0, 576), f16, kind="ExternalInput").ap()
    wp = nc.dram_tensor("wp", (P0, 384), f32, kind="ExternalInput").ap()
    dq = nc.dram_tensor("dq", (P0, 9 * P0), f16, kind="ExternalInput").ap()
    dq1 = nc.dram_tensor("dq1", (P1, 9 * P1), f16, kind="ExternalInput").ap()
    dv = nc.dram_tensor("dv", (P0, 9 * P0), f16, kind="ExternalInput").ap()
    dv1 = nc.dram_tensor("dv1", (P1, 9 * P1), f16, kind="ExternalInput").ap()
    tq = nc.dram_tensor("tq", (C, 1), f32, kind="ExternalInput").ap()
    eye = nc.dram_tensor("eye", (P0, P0), f16, kind="ExternalInput").ap()
    out = nc.dram_tensor("out", (C, HW), f32, kind="ExternalOutput").ap()
    import os
    _dbg = os.environ.get("KDBG") == "1"
    if _dbg:
        dqd = nc.dram_tensor("dqd", (P0, 8192), f32, kind="ExternalOutput").ap()
        datt = nc.dram_tensor("datt", (CHD, C), f32, kind="ExternalOutput").ap()
        dwf = nc.dram_tensor("dwf", (P0, 384), f32, kind="ExternalOutput").ap()
        dvdw = nc.dram_tensor("dvdw", (P0, 1024), f32, kind="ExternalOutput").ap()
        dgram = nc.dram_tensor("dgram", (CHD, C), f32, kind="ExternalOutput").ap()
        dqdT = nc.dram_tensor("dqdT", (P0, C), f32, kind="ExternalOutput").ap()

    import contextlib

    with tile.TileContext(nc) as tc:
      with (tc.For_i(0, iters, 1) if iters > 1 else contextlib.nullcontext()):
        with (
            tc.tile_pool(name="const", bufs=1) as cp,
            tc.tile_pool(name="band", bufs=2) as bp,
            tc.tile_pool(name="work", bufs=2) as wkp,
            tc.tile_pool(name="psA", bufs=2, space="PSUM") as psA,
            tc.tile_pool(name="psH", bufs=2, space="PSUM") as psH,
        ):
            # ---- constants ----
            # pointwise weight lhsT: [:, 0:192] = W^T rows 0:128 (K-chunk 0);
            # [0:64, 192:384] = W^T rows 128:192 (K-chunk 1).  576 = 192+384.
            wq_sb = cp.tile([P0, 576], f16)
            wv_sb = cp.tile([P0, 576], f16)
            wp_sb = cp.tile([P0, 384], f32)   # WpT rows 0:128 | rows 128:192
            dq_sb = cp.tile([P0, 9 * P0], f16)
            dq1_sb = cp.tile([P1, 9 * P1], f16)
            dv_sb = cp.tile([P0, 9 * P0], f16)
            dv1_sb = cp.tile([P1, 9 * P1], f16)
            tq_sb = cp.tile([P0, 2], f32)     # [:,0]=ch0..127, [0:64,1]=ch128..191
            eye_sb = cp.tile([P0, P0], f16)
            qd_sb = cp.tile([P0, 8192], f16)  # qd: [:,0:4096] | [0:64,4096:8192]
            vdw_sb = cp.tile([P0, 2 * HW], f16)  # v_dw: [:,0:HW] | [0:64,HW:2HW]
            g0a = cp.tile([P0, C], f32)       # gram accumulator rows 0:128
            g1a = cp.tile([P1, C], f32)       # rows 128:192
            srow = cp.tile([P0, C], f32)      # s_d broadcast to all partitions
            wf_sb = cp.tile([P0, 384], f16)   # WfT rows 0:128 | [0:64,192:384] rows 128:192
            A0 = cp.tile([P0, C], f32)        # blockdiag(attn) rows 0:128
            A1 = cp.tile([P1, C], f32)        # rows 128:192
            ssq = cp.tile([P0, 2 * NB], f32)  # row sum-of-squares per band (cols 0:8 ch0, 8:16 ch1)
            att = cp.tile([CHD, C], f32)      # per-head attn blocks, compact
            sm8 = cp.tile([CHD, 4 * HEADS], f32)  # softmax stats: max | sum | recip
            rn = cp.tile([P0, 2], f32)        # 1/||q|| * sqrt(temp)
            scr = cp.tile([P0, SUB], f32)     # scratch for sumsq STT

            nc.sync.dma_start(out=wq_sb[:, 0:384], in_=wq[:, 0:384])
            nc.sync.dma_start(out=wq_sb[0:P1, 384:576], in_=wq[0:P1, 384:576])
            nc.sync.dma_start(out=wv_sb[:, 0:384], in_=wv[:, 0:384])
            nc.sync.dma_start(out=wv_sb[0:P1, 384:576], in_=wv[0:P1, 384:576])
            nc.sync.dma_start(out=wp_sb[:, 0:192], in_=wp[:, 0:192])
            nc.sync.dma_start(out=wp_sb[0:P1, 192:384], in_=wp[0:P1, 192:384])
            nc.sync.dma_start(out=dq_sb[:], in_=dq[:])
            nc.sync.dma_start(out=dq1_sb[:], in_=dq1[:])
            nc.sync.dma_start(out=dv_sb[:], in_=dv[:])
            nc.sync.dma_start(out=dv1_sb[:], in_=dv1[:])
            nc.sync.dma_start(out=tq_sb[:, 0:1], in_=tq[0:P0, :])
            nc.sync.dma_start(out=tq_sb[0:P1, 1:2], in_=tq[P0:C, :])
            nc.sync.dma_start(out=eye_sb[:], in_=eye[:])

            XBC = PWR * W  # x band cols per chunk (2304)

            nc.gpsimd.memset(g0a[:], 0.0)
            nc.gpsimd.memset(g1a[:], 0.0)

            def packed_taps(o0, o1, d0_sb, d1_sb, rhs0_of, rhs1_of):
                """9-tap depthwise via 32x32 diag tile-matmuls, all 6 array
                tiles concurrent.  One start per PSUM bank (start=True clears
                the whole bank); per-element has_written handles the rest."""
                for t in range(9):
                    ts32 = slice(t * 32, (t + 1) * 32)
                    for g in range(4):
                        gp = slice(32 * g, 32 * (g + 1))
                        nc.tensor.matmul(
                            o0[gp], d0_sb[gp, ts32], rhs0_of(g),
                            start=(t == 0 and g == 0),
                            stop=(t == 8 and g == 3),
                            tile_position=(32 * g, 32 * g), skip_group_check=True)
                    for j, (rg, cg) in enumerate(((64, 0), (96, 32))):
                        nc.tensor.matmul(
                            o1[32 * j:32 * (j + 1)],
                            d1_sb[rg:rg + 32, ts32], rhs1_of(j),
                            start=(t == 0 and j == 0),
                            stop=(t == 8 and j == 1),
                            tile_position=(rg, cg), skip_group_check=True)

            def dma_xband(b, xband):
                h0 = b * BAND
                r_lo = h0 - 1
                xlo, xhi = max(r_lo, 0), min(r_lo + PWR, H)
                nxc = (xhi - xlo) * W
                nc.sync.dma_start(out=xband[:, 0:nxc],
                                  in_=xb[0:P0, xlo * W:xhi * W])
                nc.sync.dma_start(out=xband[0:P1, XBC:XBC + nxc],
                                  in_=xb[P0:C, xlo * W:xhi * W])

            def pw_band(b, w_sb, pb, xband):
                """Pointwise conv of band b into padded buffer pb (both chunks)."""
                h0 = b * BAND
                r_lo = h0 - 1
                xlo = max(r_lo, 0)
                pbv0 = pb[:, 0:PBSZ].rearrange("p (r c) -> p r c", c=PBW)
                pbv1 = pb[0:P1, PBSZ:2 * PBSZ].rearrange("p (r c) -> p r c", c=PBW)
                # zero the pad columns and (at image edges) halo rows
                # (memset can't encode f32r; same bits via f32 view)
                nc.gpsimd.memset(pbv0[:, :, 0:1], 0.0)
                nc.gpsimd.memset(pbv0[:, :, 129:130], 0.0)
                nc.gpsimd.memset(pbv1[:, :, 0:1], 0.0)
                nc.gpsimd.memset(pbv1[:, :, 129:130], 0.0)
                if b == 0:
                    nc.gpsimd.memset(pbv0[:, 0, :], 0.0)
                    nc.gpsimd.memset(pbv1[:, 0, :], 0.0)
                if b == NB - 1:
                    nc.gpsimd.memset(pbv0[:, PWR - 1, :], 0.0)
                    nc.gpsimd.memset(pbv1[:, PWR - 1, :], 0.0)
                for s in range(6):  # 3-row pw subtiles
                    srow = r_lo + 3 * s
                    v0, v1 = max(srow, 0), min(srow + 3, H)
                    nr = v1 - v0
                    ncols = nr * W
                    xoff = (v0 - xlo) * W
                    lr = v0 - r_lo
                    ppw0 = psA.tile([P0, 3 * W], f32, tag="pw0")
                    ppw1 = psA.tile([P1, 3 * W], f32, tag="pw1")
                    r0 = xband[:, xoff:xoff + ncols]
                    r1 = xband[0:P1, XBC + xoff:XBC + xoff + ncols]
                    nc.tensor.matmul(ppw0[:, 0:ncols], w_sb[:, 0:P0], r0,
                                     start=True, stop=False)
                    nc.tensor.matmul(ppw0[:, 0:ncols], w_sb[0:P1, 384:512], r1,
                                     start=False, stop=True)
                    nc.tensor.matmul(ppw1[:, 0:ncols], w_sb[:, P0:192], r0,
                                     start=True, stop=False)
                    nc.tensor.matmul(ppw1[:, 0:ncols], w_sb[0:P1, 512:576], r1,
                                     start=False, stop=True)
                    pv0 = ppw0[:, 0:ncols].rearrange("p (r c) -> p r c", c=W)
                    pv1 = ppw1[:, 0:ncols].rearrange("p (r c) -> p r c", c=W)
                    nc.scalar.copy(pbv0[:, lr:lr + nr, 1:129], pv0)
                    nc.scalar.copy(pbv1[:, lr:lr + nr, 1:129], pv1)
                return pbv0, pbv1

            # ========== single band sweep: q and v paths ==========
            for b in range(NB):
                h0 = b * BAND
                xband = wkp.tile([P0, 2 * XBC], f16, tag="xband")
                dma_xband(b, xband)
                pb = bp.tile([P0, 2 * PBSZ], f16, tag="pb")
                pbv0, pbv1 = pw_band(b, wq_sb, pb, xband)
                pqd0 = psH.tile([P0, SUB], f32, tag="tap0")
                pqd1 = psH.tile([P1, SUB], f32, tag="tap1")
                o0 = pqd0[:].rearrange("p (r c) -> p r c", c=64)
                o1 = pqd1[:].rearrange("p (r c) -> p r c", c=64)
                for t, (di, dj) in enumerate(TAPS):
                    st, sp = (t == 0), (t == 8)
                    rhs0 = pbv0[:, di:di + BAND:2, dj:dj + W:2]
                    rhs1 = pbv1[:, di:di + BAND:2, dj:dj + W:2]
                    nc.tensor.matmul(o0, dq_sb[:, t * P0:(t + 1) * P0], rhs0,
                                     start=st, stop=sp)
                    nc.tensor.matmul(o1, dq1_sb[:, t * P1:(t + 1) * P1], rhs1,
                                     start=st, stop=sp)
                # row sum-of-squares of this band's qd (into per-band column)
                nc.scalar.activation(scr[:], pqd0[:], Act.Square,
                                     accum_out=ssq[:, b:b + 1])
                nc.scalar.activation(scr[0:P1, :], pqd1[:], Act.Square,
                                     accum_out=ssq[0:P1, NB + b:NB + b + 1])
                nc.scalar.copy(qd_sb[:, b * SUB:(b + 1) * SUB], pqd0[:])
                nc.scalar.copy(qd_sb[0:P1, 4096 + b * SUB:4096 + (b + 1) * SUB], pqd1[:])

                # ---- this band's gram contribution (fp16), accumulated in SBUF ----
                pgb0 = psH.tile([P0, C], f32, tag="tap0")
                pgb1 = psH.tile([P1, C], f32, tag="tap1")
                for kb in range(4):
                    kcol = b * SUB + kb * P0
                    pt0 = psA.tile([P0, P0], f16, tag="pw0")
                    pt1 = psA.tile([P0, P1], f16, tag="pw1")
                    nc.tensor.transpose(pt0[:], qd_sb[:, kcol:kcol + P0], eye_sb[:])
                    nc.tensor.transpose(pt1[:], qd_sb[0:P1, 4096 + kcol:4096 + kcol + P0],
                                        eye_sb[0:P1, 0:P1])
                    qdT = wkp.tile([P0, C], f16, tag="qdT")
                    nc.vector.tensor_copy(qdT[:, 0:P0], pt0[:])
                    nc.vector.tensor_copy(qdT[:, P0:C], pt1[:])
                    nc.tensor.matmul(pgb0[:], qdT[:, 0:P0], qdT[:],
                                     start=(kb == 0), stop=(kb == 3))
                    nc.tensor.matmul(pgb1[:], qdT[:, P0:C], qdT[:],
                                     start=(kb == 0), stop=(kb == 3))
                nc.vector.tensor_tensor(g0a[:], g0a[:], pgb0[:], Alu.add)
                nc.vector.tensor_tensor(g1a[:], g1a[:], pgb1[:], Alu.add)

                # ---- v path of the same band: pw, taps, store v_dw (fp16) ----
                pbv = bp.tile([P0, 2 * PBSZ], f16, tag="pbv")
                vbv0, vbv1 = pw_band(b, wv_sb, pbv, xband)
                for s in range(NSUB):
                    ptv0 = psH.tile([P0, SUB], f32, tag="tap0")
                    ptv1 = psH.tile([P1, SUB], f32, tag="tap1")
                    o0 = ptv0[:].rearrange("p (r c) -> p r c", c=W)
                    o1 = ptv1[:].rearrange("p (r c) -> p r c", c=W)
                    for t, (di, dj) in enumerate(TAPS):
                        st, sp = (t == 0), (t == 8)
                        rhs0 = vbv0[:, 4 * s + di:4 * s + di + 4, dj:dj + W]
                        rhs1 = vbv1[:, 4 * s + di:4 * s + di + 4, dj:dj + W]
                        nc.tensor.matmul(o0, dv_sb[:, t * P0:(t + 1) * P0], rhs0,
                                         start=st, stop=sp)
                        nc.tensor.matmul(o1, dv1_sb[:, t * P1:(t + 1) * P1], rhs1,
                                         start=st, stop=sp)
                    cs = h0 * W + s * SUB
                    nc.vector.tensor_copy(vdw_sb[:, cs:cs + SUB], ptv0[:])
                    nc.vector.tensor_copy(vdw_sb[0:P1, HW + cs:HW + cs + SUB], ptv1[:])

            # ---- row scales: rn = sqrt(temp) / ||qd_row|| ----
            # ACT Sqrt is low-precision (~4e-3); one Newton step on y=sqrt(ss):
            # y' = 0.5*(y + ss/y), then rn = temp_sqrt / y'.
            nc.vector.tensor_reduce(ssq[:, 0:1], ssq[:, 0:NB], Ax.X, Alu.add)
            nc.vector.tensor_reduce(ssq[0:P1, NB:NB + 1], ssq[0:P1, NB:2 * NB],
                                    Ax.X, Alu.add)
            for ss_ap, rn_ap, tq_ap in (
                (ssq[:, 0:1], rn[:, 0:1], tq_sb[:, 0:1]),
                (ssq[0:P1, NB:NB + 1], rn[0:P1, 1:2], tq_sb[0:P1, 1:2]),
            ):
                y = scr[0:ss_ap.shape[0], 0:1]
                yr = scr[0:ss_ap.shape[0], 1:2]
                nc.scalar.activation(y, ss_ap, Act.Sqrt)
                nc.vector.reciprocal(yr, y)                      # 1/y
                nc.vector.tensor_tensor(yr, yr, ss_ap, Alu.mult)  # ss/y
                nc.vector.tensor_tensor(y, y, yr, Alu.add)
                nc.vector.tensor_scalar_mul(y, y, 0.5)            # refined sqrt
                nc.vector.reciprocal(rn_ap, y)
                nc.vector.tensor_tensor(rn_ap, rn_ap, tq_ap, Alu.mult)

            # attn = diag(s) G diag(s): row scale by s_c, then elementwise
            # multiply by s_d replicated across partitions.
            nc.sync.dma_start(out=srow[0:1, 0:P0], in_=rn[:, 0:1])
            nc.sync.dma_start(out=srow[0:1, P0:C], in_=rn[0:P1, 1:2])
            nc.gpsimd.partition_broadcast(srow[:], srow[0:1, :])
            nc.vector.tensor_scalar_mul(g0a[:], g0a[:], rn[:, 0:1])
            nc.vector.tensor_scalar_mul(g1a[:], g1a[:], rn[0:P1, 1:2])
            nc.vector.tensor_tensor(g0a[:], g0a[:], srow[:], Alu.mult)
            nc.vector.tensor_tensor(g1a[:], g1a[:], srow[0:P1, :], Alu.mult)

            # ---- extract per-head diag blocks to compact (24, 8*24) via DMA
            # (engine APs need 32-aligned partition bases; DMA does not) ----
            for h in range(HEADS):
                c0 = h * CHD
                cs = slice(c0, c0 + CHD)
                dst = att[:, cs]
                if c0 + CHD <= P0:
                    nc.sync.dma_start(out=dst, in_=g0a[cs, cs])
                elif c0 >= P0:
                    nc.sync.dma_start(out=dst, in_=g1a[c0 - P0:c0 - P0 + CHD, cs])
                else:
                    n0 = P0 - c0
                    nc.sync.dma_start(out=att[0:n0, cs], in_=g0a[c0:P0, cs])
                    nc.sync.dma_start(out=att[n0:CHD, cs],
                                      in_=g1a[0:CHD - n0, cs])
            if _dbg:
                nc.sync.dma_start(out=dgram, in_=att[:])

            # ---- softmax over d within each head block (compact layout) ----
            attv = att[:].rearrange("p (h c) -> p h c", c=CHD)
            mx = sm8[:, 0:HEADS]
            nc.vector.tensor_reduce(mx, attv, Ax.X, Alu.max)
            nc.vector.tensor_tensor(attv, attv,
                                    mx.unsqueeze(2).broadcast_to((CHD, HEADS, CHD)),
                                    Alu.subtract)
            nc.scalar.activation(att[:], att[:], Act.Exp)
            sm = sm8[:, HEADS:2 * HEADS]
            nc.vector.tensor_reduce(sm, attv, Ax.X, Alu.add)
            rs = sm8[:, 2 * HEADS:3 * HEADS]
            nc.vector.reciprocal(rs, sm)
            nc.vector.tensor_tensor(attv, attv,
                                    rs.unsqueeze(2).broadcast_to((CHD, HEADS, CHD)),
                                    Alu.mult)

            if _dbg:
                nc.sync.dma_start(out=datt, in_=att[:])
            # ---- blockdiag(A) scatter + WfT = A_bd^T-contraction with WpT ----
            nc.gpsimd.memset(A0[:], 0.0)
            nc.gpsimd.memset(A1[:], 0.0)
            for h in range(HEADS):
                c0 = h * CHD
                cs = slice(c0, c0 + CHD)
                srcb = att[:, cs]
                if c0 + CHD <= P0:
                    nc.sync.dma_start(out=A0[cs, cs], in_=srcb)
                elif c0 >= P0:
                    nc.sync.dma_start(out=A1[c0 - P0:c0 - P0 + CHD, cs], in_=srcb)
                else:  # head straddling the 128 boundary
                    n0 = P0 - c0
                    nc.sync.dma_start(out=A0[c0:P0, cs], in_=srcb[0:n0, :])
                    nc.sync.dma_start(out=A1[0:CHD - n0, cs], in_=srcb[n0:CHD, :])
            pwf0 = psH.tile([P0, C], f32, tag="tap0")
            pwf1 = psH.tile([P1, C], f32, tag="tap1")
            nc.tensor.matmul(pwf0[:], A0[:, 0:P0], wp_sb[:, 0:192], start=True, stop=False)
            nc.tensor.matmul(pwf0[:], A1[:, 0:P0], wp_sb[0:P1, 192:384], start=False, stop=True)
            nc.tensor.matmul(pwf1[:], A0[:, P0:C], wp_sb[:, 0:192], start=True, stop=False)
            nc.tensor.matmul(pwf1[:], A1[:, P0:C], wp_sb[0:P1, 192:384], start=False, stop=True)
            nc.scalar.copy(wf_sb[:, 0:192], pwf0[:])
            nc.scalar.copy(wf_sb[0:P1, 192:384], pwf1[:])

            if _dbg:
                nc.sync.dma_start(out=dwf, in_=wf_sb[:])
            # ========== final sweep: out = WfT-contraction @ v_dw ==========
            for i in range(HW // SUB):
                pool, tg = (psH, "tap") if i % 2 == 0 else (psA, "pw")
                po0 = pool.tile([P0, SUB], f32, tag=tg + "0")
                po1 = pool.tile([P1, SUB], f32, tag=tg + "1")
                r0 = vdw_sb[:, i * SUB:(i + 1) * SUB]
                r1 = vdw_sb[0:P1, HW + i * SUB:HW + (i + 1) * SUB]
                nc.tensor.matmul(po0[:], wf_sb[:, 0:P0], r0, start=True, stop=False)
                nc.tensor.matmul(po0[:], wf_sb[0:P1, 192:320], r1, start=False, stop=True)
                nc.tensor.matmul(po1[:], wf_sb[:, P0:192], r0, start=True, stop=False)
                nc.tensor.matmul(po1[:], wf_sb[0:P1, 320:384], r1, start=False, stop=True)
                ost0 = wkp.tile([P0, SUB], f32, tag="ost0")
                ost1 = wkp.tile([P1, SUB], f32, tag="ost1")
                nc.scalar.copy(ost0[:], po0[:])
                nc.vector.tensor_copy(ost1[:], po1[:])
                nc.sync.dma_start(out=out[0:P0, i * SUB:(i + 1) * SUB], in_=ost0[:])
                nc.sync.dma_start(out=out[P0:C, i * SUB:(i + 1) * SUB], in_=ost1[:])

    nc.compile()
    return nc


def _host_inputs(x, w_qkv, w_dw, w_proj, temperature):
    """Per-core input maps (host-side precompute of all weight transforms)."""
    f = np.float32
    W_q = w_qkv[0:C].astype(f)           # (192,192)
    W_v = w_qkv[2 * C:3 * C].astype(f)
    wq_d = w_dw[0:C, 0].reshape(C, 9).astype(f)        # (192,9) taps (di,dj)
    wv_d = w_dw[2 * C:3 * C, 0].reshape(C, 9).astype(f)

    def pack_T(Wm):
        """W^T chunks packed as (128, 576): [:,0:192]=rows0:128, [0:64,192:384]... wait
        layout: [:, 0:384] holds K-chunk0 (rows 0:128 of W^T) twice?  No:
        [:, 0:192] = W^T[0:128, :] ... and M-slices index cols."""
        out = np.zeros((P0, 576), f)
        WT = Wm.T.astype(f)              # (c_in, o)
        out[:, 0:192] = WT[0:P0]
        out[0:P1, 384:576] = WT[P0:C]
        return out

    def pack_diag(wd, lo, n):
        out = np.zeros((n, 9 * n), f)
        for t in range(9):
            np.fill_diagonal(out[:, t * n:(t + 1) * n], wd[lo:lo + n, t])
        return out

    wp_pack = np.zeros((P0, 384), f)
    WpT = w_proj.T.astype(f)
    wp_pack[:, 0:192] = WpT[0:P0]
    wp_pack[0:P1, 192:384] = WpT[P0:C]

    tq = np.sqrt(np.repeat(temperature.reshape(HEADS).astype(f), CHD)).reshape(C, 1)
    eye = np.eye(P0, dtype=np.float16)

    shared = {
        "wq": pack_T(W_q), "wv": pack_T(W_v), "wp": wp_pack,
        "dq": pack_diag(wq_d, 0, P0), "dq1": pack_diag(wq_d, P0, P1),
        "dv": pack_diag(wv_d, 0, P0), "dv1": pack_diag(wv_d, P0, P1),
        "tq": tq, "eye": eye,
    }
    h = np.float16
    for k in ("wq", "wv", "dq", "dq1", "dv", "dv1"):
        shared[k] = shared[k].astype(h)
    maps = []
    for b in range(8):
        m = dict(shared)
        m["xb"] = np.ascontiguousarray(x[b].reshape(C, HW).astype(h))
        maps.append(m)
    return maps


def kernel(x, w_qkv, w_dw, w_proj, temperature, _trace=False, _iters=1):
    from concourse.bass_utils import run_bass_kernel_spmd
    if _iters not in _BUILT:
        _BUILT[_iters] = _build(_iters)
    nc = _BUILT[_iters]
    in_maps = _host_inputs(
        np.asarray(x), np.asarray(w_qkv), np.asarray(w_dw),
        np.asarray(w_proj), np.asarray(temperature))
    res = run_bass_kernel_spmd(nc, in_maps, list(range(8)), trace=_trace)
    outs = [res.results[i]["out"].reshape(C, H, W) for i in range(8)]
    y = np.stack(outs, axis=0).astype(np.float32)
    kernel.last_result = res
    return y

